# revision 1
# baseline (speedup 1.0000x reference)
"""Trainium2 Bass kernel: 4096x4096 valid cross-correlation with an 11x11
filter + scalar bias, sharded column-wise across 8 NeuronCores.

Strategy
--------
Host-side sharding (halo = overlapping column slices, no collectives):
core c gets input columns [512c, 512c + 522) (core 7 shifted left to stay
in bounds) and produces output columns [512c, 512c + 512).

Per-core compute: conv expressed as banded matmuls on the TensorEngine.
For each kernel column dj, a banded stationary matrix
    B_dj[k, m] = w[k - m, dj]   (0 <= k - m < 11)
contracts over 128 image rows, while column-shifted slices of the image
slab stream as the moving operand:
    out[m, n] += sum_k B_dj[k, m] * x[r0 + k, n + dj]
Accumulating the 11 dj-shifted matmuls in one PSUM bank yields the full
11x11 correlation for a [118, 512] output tile.

Column-split beats row-split because the 4086-row extent divides into
118-row slabs with ~1% waste (35 slabs) instead of the 13% waste of
512-row shards (5 slabs incl. a 40-row tail that still streams full
width). Operands are bf16 (same 1 PE row/cycle as float32r, half the
DMA/SBUF; quantization rel err ~2.4e-3 « 2e-2 gate). PSUM accumulation
stays fp32.
"""

import os
import sys

import numpy as np

for _p in ("/opt/trn_rl_repo", "/root/.axon_site/_ro/trn_rl_repo"):
    if os.path.isdir(_p) and _p not in sys.path:
        sys.path.insert(0, _p)

# The device run goes through jax's axon PJRT backend; make sure it is
# visible if jax has not been initialized yet.
_jp = os.environ.get("JAX_PLATFORMS", "")
if "axon" not in _jp.split(","):
    os.environ["JAX_PLATFORMS"] = ("axon," + _jp).strip(",")

import ml_dtypes

import concourse.bacc as bacc
import concourse.bass as bass
import concourse.mybir as mybir
import concourse.tile as tile
from concourse.bass_utils import run_bass_kernel_spmd

H = W = 4096
KH = KW = 11
OH = OW = H - KH + 1  # 4086
NCORES = 8
COLS_OUT = 512            # output columns per core
COLS_IN = COLS_OUT + KW - 1  # 522
M_FULL = 118              # output rows per full slab (contraction K = 128)
# (x row offset, out row offset, M out rows, band column offset) per slab.
# 34 full slabs + a 74-row tail that reads the last 128 image rows and
# picks the shifted band columns 44..117.
SLABS = [(118 * s, 118 * s, 118, 0) for s in range(34)]
SLABS.append((H - 128, 34 * 118, OH - 34 * 118, 128 - (H - 34 * 118)))
assert SLABS[-1] == (3968, 4012, 74, 44)

_cache: dict = {}
LAST_RESULT = None  # BassKernelResults of the most recent device run


def _build():
    f32 = mybir.dt.float32
    bf16 = mybir.dt.bfloat16
    nc = bacc.Bacc("TRN2", target_bir_lowering=False, debug=False,
                   num_devices=NCORES)
    xs_d = nc.dram_tensor("xs", [H, COLS_IN], bf16, kind="ExternalInput")
    bd_d = nc.dram_tensor("bands", [128, KW * M_FULL], bf16,
                          kind="ExternalInput")
    bias_d = nc.dram_tensor("biasv", [1, 1], f32, kind="ExternalInput")
    out_d = nc.dram_tensor("out", [OH, COLS_OUT], f32, kind="ExternalOutput")

    with tile.TileContext(nc) as tc:
        with (
            tc.tile_pool(name="bp", bufs=1) as bp,
            tc.tile_pool(name="xp", bufs=1) as xp,
            tc.tile_pool(name="op", bufs=6) as op,
            tc.tile_pool(name="pp", bufs=6, space=bass.MemorySpace.PSUM) as pp,
            tc.tile_pool(name="pw", bufs=1, space=bass.MemorySpace.PSUM) as pw,
        ):
            # startup order matters: there is a fixed ~7 us queue-bootstrap
            # preamble before the first DMA trigger can fire, each sync
            # trigger costs ~0.7 us, and the first real matmul waits on
            # slab 0 + the dj=0..2 bands. So: slab 0 first, then the bands
            # in two pieces (the chain consumes dj in order while the rest
            # of the bands lands), bias via SWDGE off the critical queue.
            xts = [xp.tile([128, COLS_IN], bf16, tag=f"xt{si}",
                           name=f"xt{si}") for si in range(len(SLABS))]
            nc.scalar.dma_start(xts[0][:], xs_d.ap()[0:128, :])
            bt = bp.tile([128, KW * M_FULL], bf16, name="bt")
            BSPLIT = 3 * M_FULL
            nc.sync.dma_start(bt[:, 0:BSPLIT], bd_d.ap()[:, 0:BSPLIT])
            nc.sync.dma_start(bt[:, BSPLIT:], bd_d.ap()[:, BSPLIT:])
            bias_sb = bp.tile([1, 1], f32, name="bias_sb")
            nc.gpsimd.dma_start(bias_sb[:], bias_d.ap()[:, :])
            ones_t = bp.tile([1, 128], f32, name="ones_t")
            nc.gpsimd.memset(ones_t[:], 1.0)

            PREFETCH = 8
            for si in range(1, PREFETCH):
                r0 = SLABS[si][0]
                nc.scalar.dma_start(xts[si][:], xs_d.ap()[r0:r0 + 128, :])

            # No PE warmup: the HAM clock-gate ramp is a ~3.4 us time budget
            # that elapses whether the PE runs garbage or real work, so the
            # first few real matmuls may run at 1.2 GHz; a warmup prefix
            # would cost more wall time than it saves. The bias broadcast
            # matmul (K=1 against a ones row; a 128-packet broadcast DMA
            # would be ~13us) is emitted after slab 0's chain below so the
            # in-order PE queue doesn't stall on the gpsimd memset.
            bias_ps = pw.tile([128, 1], f32, name="bias_ps")
            bias_bc = bp.tile([128, 1], f32, name="bias_bc")

            # DMA economics (measured): each HWDGE DIRECT2D trigger costs
            # ~0.6-1.3 us on its issuing sequencer, and one SBUF->DRAM
            # InstDMACopy binds to ~2 SDMA engines no matter its size
            # (~50 GB/s), while separate concurrent copies grab separate
            # engines. So split each slab store in two chunks, one
            # triggered from sync and one from scalar: ~4 copies in
            # flight => ~100 GB/s of store bandwidth, ~26 us of trigger
            # time per sequencer.
            for si, (r0, o0, M, boff) in enumerate(SLABS):
                xt = xts[si]
                pt = pp.tile([M, 512], f32, tag="ps", name=f"ps{si}")
                for dj in range(KW):
                    nc.tensor.matmul(
                        pt[:, :],
                        bt[:, dj * M_FULL + boff: dj * M_FULL + boff + M],
                        xt[:, dj: dj + COLS_OUT],
                        start=(dj == 0),
                        stop=(dj == KW - 1),
                    )
                if si == 0:
                    nc.tensor.matmul(bias_ps[:], ones_t[:], bias_sb[:],
                                     start=True, stop=True)
                    nc.scalar.copy(bias_bc[:], bias_ps[:])
                ot = op.tile([M, COLS_OUT], f32, tag="ot", name=f"ot{si}")
                nc.scalar.activation(
                    ot[:, :], pt[:, :],
                    mybir.ActivationFunctionType.Identity,
                    bias=bias_bc[0:M, :],
                )
                # hybrid store: SWDGE (gpsimd) spreads over all 16 SDMA
                # engines but its Q7 descriptor generation runs ~2 us per
                # full slab - just under the slab period. Peel off 15-row
                # HWDGE chunks on sync (15-descriptor sync copies fan out;
                # bigger ones, and anything on the scalar ring, bind to 1-2
                # engines), more of them once input prefetch triggers run
                # out and at the very end so the final slabs drain over
                # many engines at once.
                nchunk = 2
                if si >= len(SLABS) - 3:
                    nchunk = 4
                g0 = 15 * nchunk
                for c0 in range(0, g0, 15):
                    c1 = min(c0 + 15, M)
                    if c0 >= c1:
                        continue
                    nc.sync.dma_start(out_d.ap()[o0 + c0:o0 + c1, :],
                                      ot[c0:c1])
                if g0 < M:
                    nc.gpsimd.dma_start(out_d.ap()[o0 + g0:o0 + M, :],
                                        ot[g0:M])
                if si + PREFETCH < len(SLABS):
                    r0n = SLABS[si + PREFETCH][0]
                    xtn = xts[si + PREFETCH]
                    nc.scalar.dma_start(xtn[:], xs_d.ap()[r0n:r0n + 128, :])
    nc.compile()
    return nc


def _bands_from_weight(weight: np.ndarray) -> np.ndarray:
    b = np.zeros((128, KW * M_FULL), np.float32)
    for dj in range(KW):
        col = weight[:, dj].astype(np.float32)
        for m in range(M_FULL):
            b[m:m + KH, dj * M_FULL + m] = col
    return b.astype(ml_dtypes.bfloat16)


def kernel(x: np.ndarray, weight: np.ndarray, bias: np.ndarray,
           _trace: bool = False, **_trace_kwargs) -> np.ndarray:
    global LAST_RESULT
    x = np.asarray(x, dtype=np.float32)
    weight = np.asarray(weight, dtype=np.float32)
    bias_v = np.asarray(bias, dtype=np.float32).reshape(1, 1)

    if "nc" not in _cache:
        _cache["nc"] = _build()
    nc = _cache["nc"]

    bands = _bands_from_weight(weight)
    xb = x.astype(ml_dtypes.bfloat16)
    starts = [min(c * COLS_OUT, W - COLS_IN) for c in range(NCORES)]
    in_maps = [
        {"xs": np.ascontiguousarray(xb[:, s:s + COLS_IN]),
         "bands": bands,
         "biasv": bias_v}
        for s in starts
    ]
    res = run_bass_kernel_spmd(nc, in_maps, core_ids=list(range(NCORES)),
                               trace=_trace, **_trace_kwargs)
    LAST_RESULT = res

    out = np.empty((OH, OW), dtype=np.float32)
    for c, s in enumerate(starts):
        r = res.results[c]["out"]
        g0 = c * COLS_OUT          # first global output col wanted from core c
        keep0 = g0 - s             # 0 for cores 0-6, 10 for core 7
        take = min(COLS_OUT - keep0, OW - g0)
        out[:, g0:g0 + take] = r[:, keep0:keep0 + take]
    return out



# revision 3
# speedup vs baseline: 1.0392x; 1.0392x over previous
"""Trainium2 Bass kernel: 4096x4096 valid cross-correlation with an 11x11
filter + scalar bias, sharded column-wise across 8 NeuronCores.

Strategy (v2: 32x32 PE array packing)
-------------------------------------
Host-side sharding as before: core c gets input columns [512c, 512c+522)
(core 7 shifted left), produces output columns [512c, 512c+512).

Per-core compute reformulated for 16-way TensorE tiling: the 128x128 PE
array is packed as 16 independent 32x32 tiles (tile_position=(32r, 32c)).
Each tile contracts K=32 image rows against a banded stationary
    B_dj[k, 32dj + m] = w[k - m, dj]   (0 <= k - m < 11, m < 22)
producing 22 output rows; 11 dj-shifted matmuls accumulate one PSUM
quadrant. Array utilization rises from 11/128 (full-array band) to
22*11/1024, cutting the streamed-column floor from ~197k to ~65k cycles.

A wave = 88 output rows = 4 blocks of 22. Block r of wave w lives in SBUF
partition group r (input rows 88w+22r .. +32, 10-row halo overlap between
groups) and computes on tile (r, c=(r+w)%4); the rotation makes 4
consecutive waves occupy all 16 tiles concurrently. Each wave accumulates
in one PSUM bank (4 partition quadrants), then ACT/DVE (alternating)
copy PSUM->SBUF as bf16. Output is stored to DRAM in the permuted
[partition, wave, col] layout and unpermuted on the host (pure index
gather + scalar bias add).

The whole per-core input (47 waves x 522 cols bf16 = 49KB/partition) and
output (48KB/partition) stay resident in SBUF, so I/O is a handful of
megabyte-scale DMAs instead of per-slab triggers: 6 overlapped-row input
loads (4-dim APs), 6 output stores, all >= 0.5MB.
"""

import os
import sys

import numpy as np

for _p in ("/opt/trn_rl_repo", "/root/.axon_site/_ro/trn_rl_repo"):
    if os.path.isdir(_p) and _p not in sys.path:
        sys.path.insert(0, _p)

# The device run goes through jax's axon PJRT backend; make sure it is
# visible if jax has not been initialized yet.
_jp = os.environ.get("JAX_PLATFORMS", "")
if "axon" not in _jp.split(","):
    os.environ["JAX_PLATFORMS"] = ("axon," + _jp).strip(",")

import ml_dtypes

import concourse.bacc as bacc
import concourse.bass as bass
import concourse.mybir as mybir
import concourse.tile as tile
from concourse.bass import AP
from concourse.bass_utils import run_bass_kernel_spmd

H = W = 4096
KH = KW = 11
OH = OW = H - KH + 1  # 4086
NCORES = 8
COLS_OUT = 512            # output columns per core
COLS_IN = COLS_OUT + KW - 1  # 522
BL = 22                   # output rows per 32x32 tile (K=32 minus 10 halo)
WV = 4 * BL               # 88 output rows per wave
NWAVE = 47                # 46 full waves + overlapping tail wave
WBASE = [88 * w for w in range(46)] + [OH - WV]  # tail at 3998
XF = COLS_IN              # free elements per wave in xAll
OF = COLS_OUT             # free elements per wave in otAll

_cache: dict = {}
LAST_RESULT = None  # BassKernelResults of the most recent device run

# host unpermute maps: output row -> (source partition, source wave)
_SRC_P = np.empty(OH, np.int64)
_SRC_W = np.empty(OH, np.int64)
for _w in range(NWAVE):
    _base = WBASE[_w]
    for _r in range(4):
        _c = (_r + _w) % 4
        _rows = np.arange(_base + BL * _r, _base + BL * _r + BL)
        _SRC_P[_rows] = 32 * _c + np.arange(BL)
        _SRC_W[_rows] = _w


def _build():
    f32 = mybir.dt.float32
    bf16 = mybir.dt.bfloat16
    nc = bacc.Bacc("TRN2", target_bir_lowering=False, debug=False,
                   num_devices=NCORES)
    xs_d = nc.dram_tensor("xs", [H, COLS_IN], bf16, kind="ExternalInput")
    bd_d = nc.dram_tensor("bands", [128, KW * 32], bf16,
                          kind="ExternalInput")
    op_d = nc.dram_tensor("operm", [128, NWAVE * OF], bf16,
                          kind="ExternalOutput")

    with tile.TileContext(nc) as tc:
        with (
            tc.tile_pool(name="bp", bufs=1) as bp,
            tc.tile_pool(name="xp", bufs=1) as xp,
            tc.tile_pool(name="op", bufs=1) as op,
            tc.tile_pool(name="pp", bufs=7, space=bass.MemorySpace.PSUM) as pp,
        ):
            xAll = xp.tile([128, NWAVE * XF], bf16, name="xAll")
            otAll = op.tile([128, NWAVE * OF], bf16, name="otAll")
            bt = bp.tile([128, KW * 32], bf16, name="bt")

            # bands first (first matmul needs them), then input chunks in
            # compute order. Overlapped-row source APs: partition group g
            # of wave w holds image rows 88w+22g .. +32.
            nc.sync.dma_start(bt[:], bd_d.ap()[:, :])
            # DMA AP balancing caps at 3 dims, so split the overlapped-row
            # input loads per partition group g: src walks (row, wave, col)
            # against a [32, n*522] SBUF destination slice.
            in_chunks = [(0, 8), (8, 24), (24, 46)]
            for (w0, w1) in in_chunks:
                n = w1 - w0
                for g in range(4):
                    src = AP(xs_d, (88 * w0 + BL * g) * COLS_IN,
                             [(COLS_IN, 32), (88 * COLS_IN, n),
                              (1, COLS_IN)])
                    eng = nc.sync if g < 2 else nc.scalar
                    eng.dma_start(xAll[32 * g:32 * g + 32, XF * w0:XF * w1],
                                  src)
            src = AP(xs_d, WBASE[46] * COLS_IN,
                     [(BL * COLS_IN, 4), (COLS_IN, 32), (1, COLS_IN)])
            nc.sync.dma_start(xAll[:, XF * 46:XF * 47], src)

            # store chunks: emitted as soon as their wave range is done so
            # stores overlap compute; both sides contiguous per partition.
            st_bounds = [0, 8, 16, 24, 32, 40, NWAVE]
            st_next = 1

            groups = [list(range(4 * g, min(4 * g + 4, NWAVE)))
                      for g in range((NWAVE + 3) // 4)]
            for waves in groups:
                pts = {w: pp.tile([128, OF], f32, tag="ps", name=f"ps{w}")
                       for w in waves}
                # (dj, w, r) order: consecutive instructions hit distinct
                # tiles so the in-order PE queue keeps 16 streams running.
                for dj in range(KW):
                    for w in waves:
                        for r in range(4):
                            c = (r + w) % 4
                            nc.tensor.matmul(
                                pts[w][32 * c:32 * c + 32, :],
                                bt[32 * r:32 * r + 32,
                                   32 * dj:32 * dj + 32],
                                xAll[32 * r:32 * r + 32,
                                     XF * w + dj:XF * w + dj + COLS_OUT],
                                start=(dj == 0),
                                stop=(dj == KW - 1),
                                tile_position=(32 * r, 32 * c),
                            )
                for w in waves:
                    if w % 2 == 0:
                        nc.scalar.copy(otAll[:, OF * w:OF * w + OF],
                                       pts[w][:, :])
                    else:
                        nc.vector.tensor_copy(otAll[:, OF * w:OF * w + OF],
                                              pts[w][:, :])
                done = waves[-1] + 1
                while st_next < len(st_bounds) and st_bounds[st_next] <= done:
                    w0, w1 = st_bounds[st_next - 1], st_bounds[st_next]
                    nc.gpsimd.dma_start(op_d.ap()[:, OF * w0:OF * w1],
                                        otAll[:, OF * w0:OF * w1])
                    st_next += 1
    nc.compile()
    return nc


def _bands_from_weight(weight: np.ndarray) -> np.ndarray:
    b = np.zeros((128, KW * 32), np.float32)
    for r in range(4):
        for dj in range(KW):
            for m in range(BL):
                b[32 * r + m:32 * r + m + KH, 32 * dj + m] = weight[:, dj]
    return b.astype(ml_dtypes.bfloat16)


def kernel(x: np.ndarray, weight: np.ndarray, bias: np.ndarray,
           _trace: bool = False, **_trace_kwargs) -> np.ndarray:
    global LAST_RESULT
    x = np.asarray(x, dtype=np.float32)
    weight = np.asarray(weight, dtype=np.float32)
    bias_v = float(np.asarray(bias, dtype=np.float32).reshape(-1)[0])

    if "nc" not in _cache:
        _cache["nc"] = _build()
    nc = _cache["nc"]

    bands = _bands_from_weight(weight)
    xb = x.astype(ml_dtypes.bfloat16)
    starts = [min(c * COLS_OUT, W - COLS_IN) for c in range(NCORES)]
    in_maps = [
        {"xs": np.ascontiguousarray(xb[:, s:s + COLS_IN]), "bands": bands}
        for s in starts
    ]
    res = run_bass_kernel_spmd(nc, in_maps, core_ids=list(range(NCORES)),
                               trace=_trace, **_trace_kwargs)
    LAST_RESULT = res

    out = np.empty((OH, OW), dtype=np.float32)
    for cc, s in enumerate(starts):
        perm = np.asarray(res.results[cc]["operm"]).reshape(128, NWAVE, OF)
        core_out = perm[_SRC_P, _SRC_W, :].astype(np.float32)
        g0 = cc * COLS_OUT          # first global output col from core cc
        keep0 = g0 - s              # 0 for cores 0-6, 10 for core 7
        take = min(COLS_OUT - keep0, OW - g0)
        out[:, g0:g0 + take] = core_out[:, keep0:keep0 + take]
    if bias_v != 0.0:
        out += bias_v
    return out
